# revision 20
# baseline (speedup 1.0000x reference)
"""Contrastive loss (NT-Xent) on 8 Trainium2 NeuronCores — v5.

Row-parallel over the [2B, 2B] similarity matrix: core c computes rows
[c*1024, (c+1)*1024). Features are L2-normalized, scaled by 256 and cast to
fp8e4m3 on the host (loss tolerance 2e-2; measured fp8 rel-err ~3e-5), then
passed host-transposed ([D, 2B]) with the column blocks rotated per core so
the diagonal / positive blocks land at fixed tile indices on every core
(uniform SPMD program). The whole [D, 2B] fp8 operand stays resident in SBUF
(64KB/partition) and serves as both the stationary and moving matmul
operand; matmuls run in fp8 DoubleRow mode into paired-bank [128,1024] PSUM
tiles so each row-sum is one wide fused exp+accumulate on the ACT engine
(keeps ACT ahead of the PE in the un-throttled window). Column-group-outer
loop order keeps the PE fed from the first 2MB of DMA onward. Each core
ships its raw per-row exp-sums (per column-group) and positive sims back to
the host, which finishes the logsumexp / mean in float64 — no device
collective, so no cross-core skew wait on the critical path.
"""

import os
import sys

for _p in ("/opt/trn_rl_repo", "/root/.axon_site/_ro/trn_rl_repo"):
    if os.path.isdir(_p) and _p not in sys.path:
        sys.path.append(_p)

import numpy as np
import ml_dtypes

B = 4096
D = 1024
TWO_B = 2 * B
TEMP = 0.07
N_CORES = 8
BLK = TWO_B // N_CORES  # 1024 rows per core
NT = TWO_B // 512  # 16 column tiles of 512
MT = BLK // 128  # 8 row tiles of 128
KT = D // 128  # 8 contraction chunks of 128
KD = KT // 2  # 4 DoubleRow chunks of 256
NG = 4  # column groups of 2048 (one pass each)
FP8_SCALE = 256.0
SCALE_EXP = 1.0 / (TEMP * FP8_SCALE * FP8_SCALE)

_cache = {}


def _build():
    import concourse.bass as bass
    import concourse.bacc as bacc
    import concourse.mybir as mybir
    from concourse.tile import TileContext

    f32 = mybir.dt.float32
    f8 = mybir.dt.float8e4
    bf16 = mybir.dt.bfloat16
    AF = mybir.ActivationFunctionType
    AX = mybir.AxisListType
    DR = mybir.MatmulPerfMode.DoubleRow

    nc = bacc.Bacc(None, target_bir_lowering=False, debug=False)
    ft8 = nc.dram_tensor("ft8", [D, TWO_B], f8, kind="ExternalInput")
    ident = nc.dram_tensor("ident", [128, 128], f32, kind="ExternalInput")
    maskinv = nc.dram_tensor("maskinv", [128, 128], f32, kind="ExternalInput")
    # rs column layout: g*16 + m*2 + jj  (jj = 1024-wide column pair index)
    rs_out = nc.dram_tensor("rs", [128, NG * MT * 2 + 1], f32, kind="ExternalOutput")
    pos_out = nc.dram_tensor("pos", [128, MT], f32, kind="ExternalOutput")

    with TileContext(nc) as tc:
        with (
            tc.tile_pool(name="ft", bufs=1) as pool_ft,
            tc.tile_pool(name="small", bufs=1) as pool_small,
            tc.tile_pool(name="exp", bufs=4) as pool_exp,
            tc.tile_pool(name="expd", bufs=2) as pool_expd,
            tc.tile_pool(name="junk", bufs=2) as pool_junk,
            tc.tile_pool(name="psum", bufs=4, space="PSUM") as psum,
            tc.tile_pool(name="dram", bufs=2, space="DRAM") as dram,
        ):
            # --- resident fp8 operand: [128, KT, 2B], element (p, k, j) =
            # ft8[k*128+p, j]. First column group split fine so the first
            # matmuls start early; issued ahead of everything else. ---
            ft_sb = pool_ft.tile([128, KT, TWO_B], f8, name="ft_sb", tag="ft_sb")
            ftr = ft8[:].rearrange("(k p) c -> p k c", p=128)
            for s in range(4):
                nc.sync.dma_start(
                    out=ft_sb[:, :, s * 512 : (s + 1) * 512],
                    in_=ftr[:, :, s * 512 : (s + 1) * 512],
                )

            # --- one tiny collective keeps the CC/comm stack initialized;
            # nothing on the compute path depends on it ---
            warm_in = dram.tile([1, 1], f32, name="warm_in")
            warm_out = dram.tile([8, 1], f32, name="warm_out")
            warm_sb = pool_small.tile([1, 1], f32, name="warm_sb", tag="warm_sb")
            nc.vector.memset(warm_sb[:], 0.0)
            nc.sync.dma_start(out=warm_in[:], in_=warm_sb[:])
            nc.gpsimd.collective_compute(
                "AllGather",
                mybir.AluOpType.bypass,
                ins=[warm_in.opt()],
                outs=[warm_out.opt()],
                replica_groups=[list(range(N_CORES))],
            )

            # --- PE warm-up: dummy back-to-back matmuls burn through the HAM
            # cold window (~3.4us) while the ft8 DMA streams in ---
            wz = pool_small.tile([128, 64], f32, name="wz", tag="wz")
            nc.gpsimd.memset(wz[:], 0.0)
            wps = psum.tile([128, 1024], f32, name="ps", tag="ps")
            for _ in range(18):
                nc.tensor.matmul(
                    wps[0:64, 0:64], wz[:, 0:64], wz[:], start=True, stop=True
                )

            ident_sb = pool_small.tile([128, 128], f32, name="ident", tag="ident")
            nc.sync.dma_start(out=ident_sb[:], in_=ident[:])
            maskinv_sb = pool_small.tile([128, 128], f32, name="maskinv", tag="maskinv")
            nc.sync.dma_start(out=maskinv_sb[:], in_=maskinv[:])
            for g in range(1, NG):
                nc.sync.dma_start(
                    out=ft_sb[:, :, g * 2048 : (g + 1) * 2048],
                    in_=ftr[:, :, g * 2048 : (g + 1) * 2048],
                )

            # --- accumulators ---
            rs_buf = pool_small.tile([128, NG * MT * 2 + 1], f32, name="rs_buf", tag="rs_buf")
            pos_all = pool_small.tile([128, MT], f32, name="pos_all", tag="pos_all")

            # --- main loop: column-group outer so the first 2MB of DMA feeds
            # 8 row tiles of matmul work (no PE starvation after ~7us) ---
            for g in range(NG):
                for m in range(MT):
                    pss = [
                        psum.tile([128, 1024], f32, name="ps", tag="ps")
                        for _ in range(2)
                    ]
                    # jj outer: pair jj's accumulation closes early so its
                    # wide exp overlaps the remaining matmuls
                    last_pair = g == NG - 1 and m == MT - 1
                    for jj in range(2):
                        for h in range(2):
                            n = g * 4 + jj * 2 + h
                            for k in range(KD):
                                nc.tensor.matmul(
                                    pss[jj][:, h * 512 : (h + 1) * 512],
                                    ft_sb[:, 2 * k : 2 * k + 2, m * 128 : (m + 1) * 128],
                                    ft_sb[:, 2 * k : 2 * k + 2, n * 512 : (n + 1) * 512],
                                    start=(k == 0),
                                    stop=(k == KD - 1),
                                    perf_mode=DR,
                                )
                            if last_pair and jj == 1:
                                # split exps so only a 512-wide exp trails the
                                # final matmul (h1 half goes to the spare col)
                                eh = pool_exp.tile([128, 512], bf16, name="exph", tag="exph")
                                nc.scalar.activation(
                                    eh[:],
                                    pss[1][:, h * 512 : (h + 1) * 512],
                                    AF.Exp,
                                    scale=SCALE_EXP,
                                    accum_out=rs_buf[:, 64:65]
                                    if h == 1
                                    else rs_buf[:, g * 16 + m * 2 + 1 : g * 16 + m * 2 + 2],
                                )
                    for jj in range(2):
                        if last_pair and jj == 1:
                            continue
                        ps = pss[jj]
                        col = g * 16 + m * 2 + jj
                        if g == 2 and jj == 0:
                            # positives: diagonal of the cross-view slab at
                            # local column m*128 (raw sim)
                            junk = pool_junk.tile([128, 128], f32, name="junk", tag="junk")
                            nc.vector.tensor_mul(
                                junk[:], ps[:, m * 128 : m * 128 + 128], ident_sb[:]
                            )
                            nc.vector.reduce_sum(
                                out=pos_all[:, m : m + 1], in_=junk[:], axis=AX.X
                            )
                        if g == 0 and jj == 0:
                            # diagonal block at local column m*128: exp, zero
                            # the self-sim, reduce on DVE (bf16: 2x rate)
                            e = pool_expd.tile([128, 1024], bf16, name="expd", tag="expd")
                            nc.scalar.activation(e[:], ps[:], AF.Exp, scale=SCALE_EXP)
                            nc.vector.tensor_mul(
                                e[:, m * 128 : m * 128 + 128],
                                e[:, m * 128 : m * 128 + 128],
                                maskinv_sb[:],
                            )
                            nc.vector.reduce_sum(
                                out=rs_buf[:, col : col + 1], in_=e[:], axis=AX.X
                            )
                        else:
                            e = pool_exp.tile([128, 1024], bf16, name="exp", tag="exp")
                            nc.scalar.activation(
                                e[:],
                                ps[:],
                                AF.Exp,
                                scale=SCALE_EXP,
                                accum_out=rs_buf[:, col : col + 1],
                            )
                # ship this pass's row sums while the next pass computes
                hi = (g + 1) * 16 + (1 if g == NG - 1 else 0)
                nc.sync.dma_start(
                    out=rs_out[:, g * 16 : hi],
                    in_=rs_buf[:, g * 16 : hi],
                )
                if g == 2:
                    nc.sync.dma_start(out=pos_out[:], in_=pos_all[:])

    nc.compile()
    return nc


def _make_in_maps(features_1: np.ndarray, features_2: np.ndarray) -> list:
    f1 = np.asarray(features_1, dtype=np.float32)
    f2 = np.asarray(features_2, dtype=np.float32)
    f = np.concatenate([f1, f2], axis=0)  # [2B, D]
    n = np.sqrt((f * f).sum(axis=1, keepdims=True, dtype=np.float32))
    fn = f / np.maximum(n, 1e-12)
    q = (fn * FP8_SCALE).astype(ml_dtypes.float8_e4m3)  # [2B, D]
    qT = np.ascontiguousarray(q.T).reshape(D, N_CORES, BLK)  # [D, 8, 1024]

    ident = np.eye(128, dtype=np.float32)
    maskinv = (1.0 - ident).astype(np.float32)

    in_maps = []
    for c in range(N_CORES):
        order = [(c + j) % N_CORES for j in range(N_CORES)]
        ft_c = np.ascontiguousarray(qT[:, order, :]).reshape(D, TWO_B)
        in_maps.append({"ft8": ft_c, "ident": ident, "maskinv": maskinv})
    return in_maps


def _host_reduce(results: list) -> np.float32:
    total = 0.0
    for c in range(N_CORES):
        rs = np.asarray(results[c]["rs"], dtype=np.float64)  # [128, NG*MT*2+1]
        po = np.asarray(results[c]["pos"], dtype=np.float64)  # [128, MT]
        rs_m = rs[:, :64].reshape(128, NG, MT, 2).sum(axis=(1, 3))  # [p, m]
        rs_m[:, MT - 1] += rs[:, 64]
        lse = np.log(rs_m)
        diff = lse - po * SCALE_EXP
        total += diff.sum()
    return np.float32(total / TWO_B)


def kernel(features_1: np.ndarray, features_2: np.ndarray) -> np.ndarray:
    from concourse.bass_utils import run_bass_kernel_spmd

    if "nc" not in _cache:
        _cache["nc"] = _build()
    nc = _cache["nc"]

    in_maps = _make_in_maps(features_1, features_2)
    res = run_bass_kernel_spmd(nc, in_maps, list(range(N_CORES)))
    return _host_reduce(res.results)


# revision 21
# speedup vs baseline: 1.0091x; 1.0091x over previous
"""Contrastive loss (NT-Xent) on 8 Trainium2 NeuronCores — v8.

Row-parallel over the [2B, 2B] similarity matrix: core c computes rows
[c*1024, (c+1)*1024). Features are L2-normalized, scaled by 256 and cast to
fp8e4m3 on the host (loss tolerance 2e-2; measured fp8 rel-err ~3e-5), then
passed host-transposed ([D, 2B]) with the column blocks rotated per core so
the diagonal / positive blocks land at fixed tile indices on every core
(uniform SPMD program). The whole [D, 2B] fp8 operand stays resident in SBUF
(64KB/partition) and serves as both the stationary and moving matmul
operand; matmuls run in fp8 DoubleRow mode into paired-bank [128,1024] PSUM
tiles so each row-sum is one wide fused exp+accumulate on the ACT engine.
The loop iterates over 1024-column pairs (outer) x row tiles (inner) so the
first 1MB of DMA already feeds 17us of matmul work (minimal early PE
starvation). Each core ships its raw per-row exp-sums (per pair) and
positive sims back to the host, which finishes the logsumexp / mean in
float64 — no device collective, so no cross-core skew wait on the critical
path.
"""

import os
import sys

for _p in ("/opt/trn_rl_repo", "/root/.axon_site/_ro/trn_rl_repo"):
    if os.path.isdir(_p) and _p not in sys.path:
        sys.path.append(_p)

import numpy as np
import ml_dtypes

B = 4096
D = 1024
TWO_B = 2 * B
TEMP = 0.07
N_CORES = 8
BLK = TWO_B // N_CORES  # 1024 rows per core
NP = TWO_B // 1024  # 8 column pairs of 1024
MT = BLK // 128  # 8 row tiles of 128
KT = D // 128  # 8 contraction chunks of 128
KD = KT // 2  # 4 DoubleRow chunks of 256
FP8_SCALE = 256.0
SCALE_EXP = 1.0 / (TEMP * FP8_SCALE * FP8_SCALE)

_cache = {}


def _build():
    import concourse.bass as bass
    import concourse.bacc as bacc
    import concourse.mybir as mybir
    from concourse.tile import TileContext

    f32 = mybir.dt.float32
    f8 = mybir.dt.float8e4
    bf16 = mybir.dt.bfloat16
    AF = mybir.ActivationFunctionType
    AX = mybir.AxisListType
    DR = mybir.MatmulPerfMode.DoubleRow

    nc = bacc.Bacc(None, target_bir_lowering=False, debug=False)
    ft8 = nc.dram_tensor("ft8", [D, TWO_B], f8, kind="ExternalInput")
    ident = nc.dram_tensor("ident", [128, 128], f32, kind="ExternalInput")
    maskinv = nc.dram_tensor("maskinv", [128, 128], f32, kind="ExternalInput")
    # rs column layout: pair*8 + m, plus one spare column for the split
    # second half of the very last pair's exp
    rs_out = nc.dram_tensor("rs", [128, NP * MT + 1], f32, kind="ExternalOutput")
    pos_out = nc.dram_tensor("pos", [128, MT], f32, kind="ExternalOutput")

    with TileContext(nc) as tc:
        with (
            tc.tile_pool(name="ft", bufs=1) as pool_ft,
            tc.tile_pool(name="small", bufs=1) as pool_small,
            tc.tile_pool(name="exp", bufs=4) as pool_exp,
            tc.tile_pool(name="expd", bufs=2) as pool_expd,
            tc.tile_pool(name="junk", bufs=2) as pool_junk,
            tc.tile_pool(name="psum", bufs=4, space="PSUM") as psum,
            tc.tile_pool(name="dram", bufs=2, space="DRAM") as dram,
        ):
            # --- resident fp8 operand: [128, KT, 2B], element (p, k, j) =
            # ft8[k*128+p, j]. First two 512-col chunks feed pair 0; the rest
            # stream in 2048-col chunks in consumption order. ---
            ft_sb = pool_ft.tile([128, KT, TWO_B], f8, name="ft_sb", tag="ft_sb")
            ftr = ft8[:].rearrange("(k p) c -> p k c", p=128)
            for s in range(4):
                nc.sync.dma_start(
                    out=ft_sb[:, :, s * 512 : (s + 1) * 512],
                    in_=ftr[:, :, s * 512 : (s + 1) * 512],
                )

            # --- one tiny collective keeps the CC/comm stack initialized;
            # nothing on the compute path depends on it ---
            warm_in = dram.tile([1, 1], f32, name="warm_in")
            warm_out = dram.tile([8, 1], f32, name="warm_out")
            warm_sb = pool_small.tile([1, 1], f32, name="warm_sb", tag="warm_sb")
            nc.vector.memset(warm_sb[:], 0.0)
            nc.sync.dma_start(out=warm_in[:], in_=warm_sb[:])
            nc.gpsimd.collective_compute(
                "AllGather",
                mybir.AluOpType.bypass,
                ins=[warm_in.opt()],
                outs=[warm_out.opt()],
                replica_groups=[list(range(N_CORES))],
            )

            # --- PE warm-up: dummy back-to-back matmuls burn through the HAM
            # cold window (~3.4us) while the ft8 DMA streams in ---
            wz = pool_small.tile([128, 64], f32, name="wz", tag="wz")
            nc.gpsimd.memset(wz[:], 0.0)
            wps = psum.tile([128, 1024], f32, name="ps", tag="ps")
            for _ in range(18):
                nc.tensor.matmul(
                    wps[0:64, 0:64], wz[:, 0:64], wz[:], start=True, stop=True
                )

            ident_sb = pool_small.tile([128, 128], f32, name="ident", tag="ident")
            nc.sync.dma_start(out=ident_sb[:], in_=ident[:])
            maskinv_sb = pool_small.tile([128, 128], f32, name="maskinv", tag="maskinv")
            nc.sync.dma_start(out=maskinv_sb[:], in_=maskinv[:])
            for g in range(1, 4):
                nc.sync.dma_start(
                    out=ft_sb[:, :, g * 2048 : (g + 1) * 2048],
                    in_=ftr[:, :, g * 2048 : (g + 1) * 2048],
                )

            # --- accumulators ---
            rs_buf = pool_small.tile([128, NP * MT + 1], f32, name="rs_buf", tag="rs_buf")
            pos_all = pool_small.tile([128, MT], f32, name="pos_all", tag="pos_all")

            # --- main loop: 1024-col pair outer, row tile inner ---
            for pr in range(NP):
                for m in range(MT):
                    ps = psum.tile([128, 1024], f32, name="ps", tag="ps")
                    last_grp = pr == NP - 1 and m == MT - 1
                    for h in range(2):
                        n = pr * 2 + h
                        for k in range(KD):
                            nc.tensor.matmul(
                                ps[:, h * 512 : (h + 1) * 512],
                                ft_sb[:, 2 * k : 2 * k + 2, m * 128 : (m + 1) * 128],
                                ft_sb[:, 2 * k : 2 * k + 2, n * 512 : (n + 1) * 512],
                                start=(k == 0),
                                stop=(k == KD - 1),
                                perf_mode=DR,
                            )
                        if last_grp:
                            # split exps so only a 512-wide exp trails the
                            # final matmul (h1 half goes to the spare col)
                            eh = pool_exp.tile([128, 512], bf16, name="exph", tag="exph")
                            nc.scalar.activation(
                                eh[:],
                                ps[:, h * 512 : (h + 1) * 512],
                                AF.Exp,
                                scale=SCALE_EXP,
                                accum_out=rs_buf[:, 64:65]
                                if h == 1
                                else rs_buf[:, pr * MT + m : pr * MT + m + 1],
                            )
                    col = pr * MT + m
                    if last_grp:
                        pass
                    elif pr == 0:
                        # diagonal block at local column m*128: exp, zero the
                        # self-sim, reduce on DVE (bf16: 2x rate)
                        e = pool_expd.tile([128, 1024], bf16, name="expd", tag="expd")
                        nc.scalar.activation(e[:], ps[:], AF.Exp, scale=SCALE_EXP)
                        nc.vector.tensor_mul(
                            e[:, m * 128 : m * 128 + 128],
                            e[:, m * 128 : m * 128 + 128],
                            maskinv_sb[:],
                        )
                        nc.vector.reduce_sum(
                            out=rs_buf[:, col : col + 1], in_=e[:], axis=AX.X
                        )
                    else:
                        if pr == 4:
                            # positives: diagonal of the cross-view slab at
                            # local column m*128 (raw sim)
                            junk = pool_junk.tile([128, 128], f32, name="junk", tag="junk")
                            nc.vector.tensor_mul(
                                junk[:], ps[:, m * 128 : m * 128 + 128], ident_sb[:]
                            )
                            nc.vector.reduce_sum(
                                out=pos_all[:, m : m + 1], in_=junk[:], axis=AX.X
                            )
                        e = pool_exp.tile([128, 1024], bf16, name="exp", tag="exp")
                        nc.scalar.activation(
                            e[:],
                            ps[:],
                            AF.Exp,
                            scale=SCALE_EXP,
                            accum_out=rs_buf[:, col : col + 1],
                        )
                # ship this pair's row sums while the next pair computes
                hi = (pr + 1) * MT + (1 if pr == NP - 1 else 0)
                nc.sync.dma_start(
                    out=rs_out[:, pr * MT : hi], in_=rs_buf[:, pr * MT : hi]
                )
                if pr == 4:
                    nc.sync.dma_start(out=pos_out[:], in_=pos_all[:])

    nc.compile()
    return nc


def _make_in_maps(features_1: np.ndarray, features_2: np.ndarray) -> list:
    f1 = np.asarray(features_1, dtype=np.float32)
    f2 = np.asarray(features_2, dtype=np.float32)
    f = np.concatenate([f1, f2], axis=0)  # [2B, D]
    n = np.sqrt((f * f).sum(axis=1, keepdims=True, dtype=np.float32))
    fn = f / np.maximum(n, 1e-12)
    q = (fn * FP8_SCALE).astype(ml_dtypes.float8_e4m3)  # [2B, D]
    qT = np.ascontiguousarray(q.T).reshape(D, N_CORES, BLK)  # [D, 8, 1024]

    ident = np.eye(128, dtype=np.float32)
    maskinv = (1.0 - ident).astype(np.float32)

    in_maps = []
    for c in range(N_CORES):
        order = [(c + j) % N_CORES for j in range(N_CORES)]
        ft_c = np.ascontiguousarray(qT[:, order, :]).reshape(D, TWO_B)
        in_maps.append({"ft8": ft_c, "ident": ident, "maskinv": maskinv})
    return in_maps


def _host_reduce(results: list) -> np.float32:
    total = 0.0
    for c in range(N_CORES):
        rs = np.asarray(results[c]["rs"], dtype=np.float64)  # [128, NP*MT+1]
        po = np.asarray(results[c]["pos"], dtype=np.float64)  # [128, MT]
        rs_m = rs[:, : NP * MT].reshape(128, NP, MT).sum(axis=1)  # [p, m]
        rs_m[:, MT - 1] += rs[:, NP * MT]
        lse = np.log(rs_m)
        diff = lse - po * SCALE_EXP
        total += diff.sum()
    return np.float32(total / TWO_B)


def kernel(features_1: np.ndarray, features_2: np.ndarray) -> np.ndarray:
    from concourse.bass_utils import run_bass_kernel_spmd

    if "nc" not in _cache:
        _cache["nc"] = _build()
    nc = _cache["nc"]

    in_maps = _make_in_maps(features_1, features_2)
    res = run_bass_kernel_spmd(nc, in_maps, list(range(N_CORES)))
    return _host_reduce(res.results)
